# revision 1
# baseline (speedup 1.0000x reference)
"""DFRNN forward kernel for TRN2 (8 NeuronCores, data-parallel over num_ts).

Reference computation (per token z = X[ts, t, :], 128 features):
  global: 3-layer LSTM single-step-from-zero stack (hidden 512)
          h = sigmoid(o) * tanh(sigmoid(i) * tanh(c)); f-gate is dead (c0=0)
  mu     = relu(h3) . sum_f(e_W) + sum(e_b)
  noise:  2-layer stack (hidden 128), st = relu(h2_n) . a_W + a_b
  sigma  = softplus(st) + 1e-6

Layout: channels on partitions, tokens on free dim. float32r matmuls
(~13-bit mantissa, full PE rate at N>=256). Weights pre-transposed and
packed on host; biases fused into ACT per-partition bias.
"""

import contextlib

import numpy as np

import concourse.bass as bass
import concourse.mybir as mybir
import concourse.tile as tile
from concourse import bacc, bass_utils
from concourse.masks import make_identity

P = 128
IN = 128
GH = 512
NH = 128
NCORES = 8
FULL_NT = 1024
T = 192

SB = 1536              # tokens per superblock
CHUNKS = SB // P       # 12
NBK = SB // 512        # 3 psum banks per gate tile

F32 = mybir.dt.float32
F16 = mybir.dt.float16
F32R = mybir.dt.float32r
AF = mybir.ActivationFunctionType
ALU = mybir.AluOpType

# tanh(x) ~= x*(C0 + C1 u + C2 u^2 + C3 u^3), u=x^2, max err 3.3e-5 on [-1,1]
TANH_C = (0.99969411, -0.3288952, 0.11542124, -0.02465895)

# ---- weight slot layout (shared by host packing and IR emission) ----
# each slot is a [128, 128] lhsT tile = W[rows, cols].T
GATES = ("i", "c", "o")


def _slot_table():
    slots = []  # (kind, l, k, gate, j)
    for l in range(3):
        K = 1 if l == 0 else 4
        for k in range(K):
            for g in GATES:
                for j in range(4):
                    slots.append(("g", l, k, g, j))
    for l in range(2):
        for g in GATES:
            slots.append(("n", l, 0, g, 0))
    idx = {s: i for i, s in enumerate(slots)}
    nslots = len(slots) + 1          # +1 for wsum/a_w slot
    return idx, len(slots), nslots


SLOT_IDX, VEC_SLOT, NSLOTS = _slot_table()
WTW = NSLOTS * P                     # wt tensor free width
NBIAS = 48                           # 36 global + 6 noise, padded


def _bias_col(kind, l, g, j):
    gi = GATES.index(g)
    if kind == "g":
        return (l * 3 + gi) * 4 + j
    return 36 + l * 3 + gi


def pack_host(g_Wih0, g_bih0, g_bhh0, g_Wih, g_bih, g_bhh, e_W, e_b,
              n_Wih0, n_bih0, n_bhh0, n_Wih, n_bih, n_bhh, a_W, a_b):
    """Host-side packing: transposed weight tiles + combined biases."""
    g_off = {"i": 0, "c": 2 * GH, "o": 3 * GH}
    n_off = {"i": 0, "c": 2 * NH, "o": 3 * NH}
    wt = np.zeros((P, WTW), np.float32)
    for (kind, l, k, g, j), si in SLOT_IDX.items():
        if kind == "g":
            W = g_Wih0 if l == 0 else g_Wih[l - 1]
            rows = slice(g_off[g] + j * P, g_off[g] + (j + 1) * P)
        else:
            W = n_Wih0 if l == 0 else n_Wih[0]
            rows = slice(n_off[g] + j * P, n_off[g] + (j + 1) * P)
        cols = slice(k * P, (k + 1) * P)
        wt[:, si * P:(si + 1) * P] = np.asarray(W)[rows, cols].T
    wsum = np.asarray(e_W).sum(axis=0)            # [GH]
    base = VEC_SLOT * P
    for j in range(4):
        wt[:, base + j] = wsum[j * P:(j + 1) * P]
    wt[:, base + 4] = np.asarray(a_W)[0]

    bias = np.zeros((P, NBIAS), np.float32)
    bg0 = np.asarray(g_bih0) + np.asarray(g_bhh0)
    bn0 = np.asarray(n_bih0) + np.asarray(n_bhh0)
    for (kind, l, g, j) in [(k, l, g, j) for (k, l, _, g, j) in SLOT_IDX]:
        if kind == "g":
            b = bg0 if l == 0 else np.asarray(g_bih[l - 1]) + np.asarray(g_bhh[l - 1])
            off = g_off[g]
        else:
            b = bn0 if l == 0 else np.asarray(n_bih[0]) + np.asarray(n_bhh[0])
            off = n_off[g]
        bias[:, _bias_col(kind, l, g, j)] = b[off + j * P: off + (j + 1) * P]

    b_sum = float(np.asarray(e_b).sum())
    a_bias = float(np.asarray(a_b)[0])
    return wt, bias, b_sum, a_bias


def build_nc(tok, b_sum, a_bias, chain=False, repeat=1):
    """Emit the per-core kernel for `tok` tokens (multiple of SB).

    chain=True adds a dummy input mixed (x0) into mu so repeated
    invocations can be data-dependent for wall-clock timing."""
    nsb = tok // SB
    nc = bacc.Bacc("TRN2", target_bir_lowering=False, debug=False)
    x = nc.dram_tensor("x", [tok, IN], F32, kind="ExternalInput").ap()
    wt_d = nc.dram_tensor("wt", [P, WTW], F32, kind="ExternalInput").ap()
    bias_d = nc.dram_tensor("bias", [P, NBIAS], F32, kind="ExternalInput").ap()
    chain_d = (nc.dram_tensor("chain", [P, nsb * CHUNKS], F32,
                              kind="ExternalInput").ap() if chain else None)
    mu_d = nc.dram_tensor("mu", [P, nsb * CHUNKS], F32, kind="ExternalOutput").ap()
    sg_d = nc.dram_tensor("sigma", [P, nsb * CHUNKS], F32, kind="ExternalOutput").ap()

    x4 = x.rearrange("(b s p) f -> b p s f", s=CHUNKS, p=P)

    with tile.TileContext(nc) as tc:
        with (
            tc.tile_pool(name="const", bufs=1) as cpool,
            tc.tile_pool(name="stage", bufs=2) as stpool,
            tc.tile_pool(name="xin", bufs=2) as xpool,
            tc.tile_pool(name="zt", bufs=2) as ztpool,
            tc.tile_pool(name="h", bufs=2) as hpool,
            tc.tile_pool(name="tmp", bufs=6) as tpool,
            tc.tile_pool(name="ps", bufs=2, space="PSUM") as pspool,
        ):
            ident = cpool.tile([P, P], F32)
            make_identity(nc, ident[:])
            wtr = cpool.tile([P, WTW], F32R)
            biast = cpool.tile([P, NBIAS], F32)
            nc.sync.dma_start(biast[:], bias_d)
            CH = 1024
            for c0 in range(0, WTW, CH):
                c1 = min(c0 + CH, WTW)
                stg = stpool.tile([P, CH], F32, tag="wstage")
                nc.sync.dma_start(stg[:, : c1 - c0], wt_d[:, c0:c1])
                nc.vector.tensor_copy(wtr[:, c0:c1], stg[:, : c1 - c0])

            musb = cpool.tile([P, nsb * CHUNKS], F32)
            stsb = cpool.tile([P, nsb * CHUNKS], F32)
            sgsb = cpool.tile([P, nsb * CHUNKS], F32)

            def wslot(kind, l, k, g, j):
                si = SLOT_IDX[(kind, l, k, g, j)]
                return wtr[:, si * P:(si + 1) * P]

            def bcol(kind, l, g, j):
                c = _bias_col(kind, l, g, j)
                return biast[:, c:c + 1]

            wsum_cols = [wtr[:, VEC_SLOT * P + j: VEC_SLOT * P + j + 1]
                         for j in range(4)]
            aw_col = wtr[:, VEC_SLOT * P + 4: VEC_SLOT * P + 5]

            rep_cm = (tc.For_i(0, repeat, 1) if repeat > 1
                      else contextlib.nullcontext())
            with rep_cm:
              for b in range(nsb):
                # ---- load + transpose X block: zt[feat, tok] ----
                xin = xpool.tile([P, CHUNKS, P], F32, tag="xin")
                nc.sync.dma_start(xin[:], x4[b])
                zt = ztpool.tile([P, SB], F32R, tag="zt")
                tp = pspool.tile([P, SB], F32, tag="gate")
                for s in range(CHUNKS):
                    nc.tensor.transpose(tp[:, s * P:(s + 1) * P],
                                        xin[:, s, :], ident[:])
                nc.vector.tensor_copy(zt[:], tp[:])

                # ---- interleaved global (3L) + noise (2L) stacks ----
                c0_, c1_, c2_, c3_ = TANH_C

                def tanh_poly(x):
                    """DVE tanh approx for |x|<1; returns a fresh tmp tile."""
                    u = tpool.tile([P, SB], F32, tag="tmp")
                    nc.vector.tensor_tensor(u[:], x, x, ALU.mult)
                    p = tpool.tile([P, SB], F32, tag="tmp")
                    nc.vector.tensor_scalar(p[:], u[:], c3_, c2_, ALU.mult, ALU.add)
                    nc.vector.tensor_tensor(p[:], p[:], u[:], ALU.mult)
                    nc.vector.tensor_scalar(p[:], p[:], 1.0, c1_, ALU.mult, ALU.add)
                    nc.vector.tensor_tensor(p[:], p[:], u[:], ALU.mult)
                    nc.vector.tensor_scalar(p[:], p[:], 1.0, c0_, ALU.mult, ALU.add)
                    nc.vector.tensor_tensor(p[:], p[:], x, ALU.mult)
                    return p

                def emit_gates(lhs_of, rhs_of, K, bias_of, hj, last):
                    """One LSTM cell tile: i,c,o gates -> hj (+relu if last)."""
                    ps_i = pspool.tile([P, SB], F32, tag="gate")
                    for nb in range(NBK):
                        for k in range(K):
                            nc.tensor.matmul(ps_i[:, nb * 512:(nb + 1) * 512],
                                             lhs_of("i", k), rhs_of(k, nb),
                                             start=(k == 0), stop=(k == K - 1))
                    ps_c = pspool.tile([P, SB], F32, tag="gate")
                    for nb in range(NBK):
                        for k in range(K):
                            nc.tensor.matmul(ps_c[:, nb * 512:(nb + 1) * 512],
                                             lhs_of("c", k), rhs_of(k, nb),
                                             start=(k == 0), stop=(k == K - 1))
                    si = tpool.tile([P, SB], F16, tag="tmp")
                    nc.scalar.activation(si[:], ps_i[:], AF.Sigmoid,
                                         bias=bias_of("i"))
                    tcv = tpool.tile([P, SB], F16, tag="tmp")
                    nc.scalar.activation(tcv[:], ps_c[:], AF.Tanh,
                                         bias=bias_of("c"))
                    nc.vector.tensor_tensor(si[:], si[:], tcv[:], ALU.mult)
                    nc.scalar.activation(si[:], si[:], AF.Tanh)
                    tcc = si
                    ps_o = pspool.tile([P, SB], F32, tag="gate")
                    for nb in range(NBK):
                        for k in range(K):
                            nc.tensor.matmul(ps_o[:, nb * 512:(nb + 1) * 512],
                                             lhs_of("o", k), rhs_of(k, nb),
                                             start=(k == 0), stop=(k == K - 1))
                    so = tpool.tile([P, SB], F16, tag="tmp")
                    nc.scalar.activation(so[:], ps_o[:], AF.Sigmoid,
                                         bias=bias_of("o"))
                    nc.vector.tensor_tensor(hj, so[:], tcc[:], ALU.mult)
                    if last:
                        nc.vector.tensor_scalar_max(hj, hj, 0.0)

                hg = [None, None, None]   # global h per layer
                hn = [None, None]         # noise h per layer

                def emit_global(l):
                    K = 1 if l == 0 else 4
                    hcur = hpool.tile([P, 4 * SB], F32R, tag="hg")
                    hprev = hg[l - 1] if l else None
                    for j in range(4):
                        def rhs(k, nb):
                            if l == 0:
                                return zt[:, nb * 512:(nb + 1) * 512]
                            return hprev[:, k * SB + nb * 512:
                                         k * SB + (nb + 1) * 512]
                        emit_gates(lambda g, k, j=j: wslot("g", l, k, g, j),
                                   rhs, K,
                                   lambda g, j=j: bcol("g", l, g, j),
                                   hcur[:, j * SB:(j + 1) * SB], l == 2)
                    hg[l] = hcur

                def emit_noise(l):
                    hcur = hpool.tile([P, SB], F32R, tag="hn")
                    src_t = zt if l == 0 else hn[0]

                    def rhs(k, nb):
                        return src_t[:, nb * 512:(nb + 1) * 512]
                    emit_gates(lambda g, k: wslot("n", l, 0, g, 0),
                               rhs, 1,
                               lambda g: bcol("n", l, g, 0),
                               hcur[:], l == 1)
                    hn[l] = hcur

                emit_global(0)
                emit_noise(0)
                emit_global(1)
                emit_noise(1)
                emit_global(2)
                gt = hg[2]
                hnf = hn[1]

                # ---- st / mu reductions: tokens land on partitions ----
                st_ps = pspool.tile([P, CHUNKS], F32, tag="sm")
                for c in range(CHUNKS):
                    nc.tensor.matmul(st_ps[:, c:c + 1],
                                     hnf[:, c * P:(c + 1) * P].bitcast(F32),
                                     aw_col.bitcast(F32), start=True, stop=True)
                nc.vector.tensor_copy(
                    stsb[:, b * CHUNKS:(b + 1) * CHUNKS], st_ps[:])
                mu_ps = pspool.tile([P, CHUNKS], F32, tag="sm")
                for c in range(CHUNKS):
                    for j in range(4):
                        nc.tensor.matmul(
                            mu_ps[:, c:c + 1],
                            gt[:, j * SB + c * P: j * SB + (c + 1) * P].bitcast(F32),
                            wsum_cols[j].bitcast(F32),
                            start=(j == 0), stop=(j == 3))
                nc.vector.tensor_scalar_add(
                    musb[:, b * CHUNKS:(b + 1) * CHUNKS], mu_ps[:], b_sum)

              if chain_d is not None:
                cht = cpool.tile([P, nsb * CHUNKS], F32)
                nc.sync.dma_start(cht[:], chain_d)
                nc.vector.tensor_scalar(cht[:], cht[:], 0.0, None, ALU.mult)
                nc.vector.tensor_tensor(musb[:], musb[:], cht[:], ALU.add)

              # ---- epilogue: softplus = ln(1 + exp(st + a_b)) ----
              nc.scalar.activation(sgsb[:], stsb[:], AF.Exp, bias=a_bias)
              nc.scalar.activation(sgsb[:], sgsb[:], AF.Ln, bias=1.0)
              nc.vector.tensor_scalar_add(sgsb[:], sgsb[:], 1e-6)
              nc.sync.dma_start(mu_d, musb[:])
              nc.sync.dma_start(sg_d, sgsb[:])

    nc.compile()
    return nc


def _unshuffle(arr, tok):
    """[P, nsb*CHUNKS] device layout -> flat [tok] token order."""
    nsb = tok // SB
    return (arr.reshape(P, nsb, CHUNKS).transpose(1, 2, 0).reshape(tok))


def run(X, weights_kwargs, tok_per_core, n_cores, trace=False):
    """Shard X over cores, run, gather. X: [nt, T, IN] with nt*T == n_cores*tok."""
    wt, bias, b_sum, a_bias = pack_host(**weights_kwargs)
    nc = build_nc(tok_per_core, b_sum, a_bias)
    Xf = np.ascontiguousarray(np.asarray(X), np.float32).reshape(-1, IN)
    in_maps = []
    for c in range(n_cores):
        shard = Xf[c * tok_per_core:(c + 1) * tok_per_core]
        in_maps.append({"x": np.ascontiguousarray(shard),
                        "wt": wt, "bias": bias})
    res = bass_utils.run_bass_kernel_spmd(
        nc, in_maps, core_ids=list(range(n_cores)), trace=trace)
    mus, sgs = [], []
    for c in range(n_cores):
        mus.append(_unshuffle(res.results[c]["mu"], tok_per_core))
        sgs.append(_unshuffle(res.results[c]["sigma"], tok_per_core))
    nt = X.shape[0]
    mu = np.concatenate(mus).reshape(nt, T).astype(np.float32)
    sg = np.concatenate(sgs).reshape(nt, T).astype(np.float32)
    return mu, sg, res


def kernel(X, g_Wih0, g_bih0, g_bhh0, g_Wih, g_bih, g_bhh, e_W, e_b,
           n_Wih0, n_bih0, n_bhh0, n_Wih, n_bih, n_bhh, a_W, a_b):
    wk = dict(g_Wih0=g_Wih0, g_bih0=g_bih0, g_bhh0=g_bhh0, g_Wih=g_Wih,
              g_bih=g_bih, g_bhh=g_bhh, e_W=e_W, e_b=e_b, n_Wih0=n_Wih0,
              n_bih0=n_bih0, n_bhh0=n_bhh0, n_Wih=n_Wih, n_bih=n_bih,
              n_bhh=n_bhh, a_W=a_W, a_b=a_b)
    tok = FULL_NT * T // NCORES      # 24576
    mu, sg, _ = run(X, wk, tok, NCORES)
    return mu, sg



# revision 10
# speedup vs baseline: 58.4975x; 58.4975x over previous
"""DFRNN forward kernel for TRN2 (8 NeuronCores, data-parallel over num_ts).

Reference computation (per token z = X[ts, t, :], 128 features):
  global: 3-layer LSTM single-step-from-zero stack (hidden 512)
          h = sigmoid(o) * tanh(sigmoid(i) * tanh(c)); f-gate is dead (c0=0)
  mu     = relu(h3) . sum_f(e_W) + sum(e_b)
  noise:  2-layer stack (hidden 128), st = relu(h2_n) . a_W + a_b
  sigma  = softplus(st) + 1e-6

Layout: channels on partitions, tokens on free dim, f16 operands with
fp32 PSUM accumulation. X is cast to f16 on host and transposed on the
fly by DMA-transpose (no PE transposes). Gate groups run o,i,c with an
early sigmoid(o) so the 2-buffer PSUM ring pipelines without stalls.
mu/st heads are weight-stationary N=512 matmuls into a shared [2,512]
PSUM bank, accumulated into an SBUF row buffer and reshaped once at the
end via a DRAM bounce.
"""

import numpy as np

import concourse.bass as bass
import concourse.mybir as mybir
import concourse.tile as tile
from concourse import bacc, bass_utils

P = 128
IN = 128
GH = 512
NH = 128
NCORES = 8
FULL_NT = 1024
T = 192

SB = 1536              # tokens per superblock
NBK = SB // 512        # psum banks per gate tile

F32 = mybir.dt.float32
F16 = mybir.dt.float16
AF = mybir.ActivationFunctionType
ALU = mybir.AluOpType

# ---- weight slot layout (shared by host packing and IR emission) ----
# each slot is a [128, 128] lhsT tile = W[rows, cols].T
GATES = ("i", "c", "o")


def _slot_table():
    slots = []  # (kind, l, k, gate, j)
    for l in range(3):
        K = 1 if l == 0 else 4
        for k in range(K):
            for g in GATES:
                for j in range(4):
                    slots.append(("g", l, k, g, j))
    for l in range(2):
        for g in GATES:
            slots.append(("n", l, 0, g, 0))
    idx = {s: i for i, s in enumerate(slots)}
    nslots = len(slots) + 1          # +1 for wsum/a_w vector slot
    return idx, len(slots), nslots


SLOT_IDX, VEC_SLOT, NSLOTS = _slot_table()
WTW = NSLOTS * P                     # wt tensor free width
NBIAS = 48                           # 36 global + 6 noise, padded


def _bias_col(kind, l, g, j):
    gi = GATES.index(g)
    if kind == "g":
        return (l * 3 + gi) * 4 + j
    return 36 + l * 3 + gi


def pack_host(g_Wih0, g_bih0, g_bhh0, g_Wih, g_bih, g_bhh, e_W, e_b,
              n_Wih0, n_bih0, n_bhh0, n_Wih, n_bih, n_bhh, a_W, a_b):
    """Host-side packing: transposed f16 weight tiles + combined biases."""
    g_off = {"i": 0, "c": 2 * GH, "o": 3 * GH}
    n_off = {"i": 0, "c": 2 * NH, "o": 3 * NH}
    wt = np.zeros((P, WTW), np.float16)
    for (kind, l, k, g, j), si in SLOT_IDX.items():
        if kind == "g":
            W = g_Wih0 if l == 0 else g_Wih[l - 1]
            rows = slice(g_off[g] + j * P, g_off[g] + (j + 1) * P)
        else:
            W = n_Wih0 if l == 0 else n_Wih[0]
            rows = slice(n_off[g] + j * P, n_off[g] + (j + 1) * P)
        cols = slice(k * P, (k + 1) * P)
        wt[:, si * P:(si + 1) * P] = np.asarray(W)[rows, cols].T.astype(np.float16)
    # vec slot cols: [wsum0, 0, wsum1, wsum2, wsum3, 0, a_w] — the zero
    # columns pair wsum0/a_w into 2-col lhsTs so both psum rows are
    # covered by the start=True matmul (has_written clears per element).
    wsum = np.asarray(e_W).sum(axis=0)            # [GH]
    base = VEC_SLOT * P
    wt[:, base + 0] = wsum[0:P].astype(np.float16)
    for j in range(1, 4):
        wt[:, base + 1 + j] = wsum[j * P:(j + 1) * P].astype(np.float16)
    wt[:, base + 6] = np.asarray(a_W)[0].astype(np.float16)

    bias = np.zeros((P, NBIAS), np.float32)
    bg0 = np.asarray(g_bih0) + np.asarray(g_bhh0)
    bn0 = np.asarray(n_bih0) + np.asarray(n_bhh0)
    for (kind, l, g, j) in [(k, l, g, j) for (k, l, _, g, j) in SLOT_IDX]:
        if kind == "g":
            b = bg0 if l == 0 else np.asarray(g_bih[l - 1]) + np.asarray(g_bhh[l - 1])
            off = g_off[g]
        else:
            b = bn0 if l == 0 else np.asarray(n_bih[0]) + np.asarray(n_bhh[0])
            off = n_off[g]
        bias[:, _bias_col(kind, l, g, j)] = b[off + j * P: off + (j + 1) * P]

    b_sum = float(np.asarray(e_b).sum())
    a_bias = float(np.asarray(a_b)[0])
    return wt, bias, b_sum, a_bias


def build_nc(tok, b_sum, a_bias):
    """Emit the per-core kernel for `tok` tokens (multiple of SB)."""
    nsb = tok // SB
    outw = tok // P
    nc = bacc.Bacc("TRN2", target_bir_lowering=False, debug=False)
    x = nc.dram_tensor("x", [tok, IN], F16, kind="ExternalInput").ap()
    wt_d = nc.dram_tensor("wt", [P, WTW], F16, kind="ExternalInput").ap()
    bias_d = nc.dram_tensor("bias", [P, NBIAS], F32, kind="ExternalInput").ap()
    mu_d = nc.dram_tensor("mu", [P, outw], F32, kind="ExternalOutput").ap()
    sg_d = nc.dram_tensor("sigma", [P, outw], F32, kind="ExternalOutput").ap()
    scr_mu = nc.dram_tensor("scr_mu", [tok], F32, kind="Internal").ap()
    scr_st = nc.dram_tensor("scr_st", [tok], F32, kind="Internal").ap()

    with tile.TileContext(nc) as tc:
        with (
            tc.tile_pool(name="const", bufs=1) as cpool,
            tc.tile_pool(name="zt", bufs=2) as ztpool,
            tc.tile_pool(name="hg", bufs=3) as hgpool,
            tc.tile_pool(name="hn", bufs=2) as hnpool,
            tc.tile_pool(name="tmp", bufs=6) as tpool,
            tc.tile_pool(name="gate", bufs=2, space="PSUM") as gpool,
            tc.tile_pool(name="head", bufs=2, space="PSUM") as hpool,
        ):
            wtr = cpool.tile([P, WTW], F16)
            nc.sync.dma_start(wtr[:], wt_d)
            biast = cpool.tile([P, NBIAS], F32)
            nc.sync.dma_start(biast[:], bias_d)
            rowacc = cpool.tile([2, tok], F32)

            def wslot(kind, l, k, g, j):
                si = SLOT_IDX[(kind, l, k, g, j)]
                return wtr[:, si * P:(si + 1) * P]

            def bcol(kind, l, g, j):
                c = _bias_col(kind, l, g, j)
                return biast[:, c:c + 1]

            vb = VEC_SLOT * P
            wsum0z = wtr[:, vb + 0: vb + 2]       # [wsum0, zeros]
            wsum_cols = [None] + [wtr[:, vb + 1 + j: vb + 2 + j]
                                  for j in range(1, 4)]
            aw2 = wtr[:, vb + 5: vb + 7]          # [zeros, a_w]

            def gate_group(lhs_of, rhs_of, K, bias_of, hj, last):
                """One LSTM cell tile: o,i,c gates -> hj (+relu if last)."""
                ps_o = gpool.tile([P, SB], F32, tag="gate")
                for nb in range(NBK):
                    for k in range(K):
                        nc.tensor.matmul(ps_o[:, nb * 512:(nb + 1) * 512],
                                         lhs_of("o", k), rhs_of(k, nb),
                                         start=(k == 0), stop=(k == K - 1))
                ps_i = gpool.tile([P, SB], F32, tag="gate")
                for nb in range(NBK):
                    for k in range(K):
                        nc.tensor.matmul(ps_i[:, nb * 512:(nb + 1) * 512],
                                         lhs_of("i", k), rhs_of(k, nb),
                                         start=(k == 0), stop=(k == K - 1))
                so = tpool.tile([P, SB], F16, tag="tmp")
                nc.scalar.activation(so[:], ps_o[:], AF.Sigmoid,
                                     bias=bias_of("o"))
                ps_c = gpool.tile([P, SB], F32, tag="gate")
                for nb in range(NBK):
                    for k in range(K):
                        nc.tensor.matmul(ps_c[:, nb * 512:(nb + 1) * 512],
                                         lhs_of("c", k), rhs_of(k, nb),
                                         start=(k == 0), stop=(k == K - 1))
                si = tpool.tile([P, SB], F16, tag="tmp")
                nc.scalar.activation(si[:], ps_i[:], AF.Sigmoid,
                                     bias=bias_of("i"))
                tc_ = tpool.tile([P, SB], F16, tag="tmp")
                nc.scalar.activation(tc_[:], ps_c[:], AF.Tanh,
                                     bias=bias_of("c"))
                nc.vector.tensor_tensor(si[:], si[:], tc_[:], ALU.mult)
                nc.scalar.activation(si[:], si[:], AF.Tanh)
                if last:  # relu(so*tcc) == max(tcc,0)*so since so>0
                    nc.vector.scalar_tensor_tensor(hj, si[:], 0.0, so[:],
                                                   ALU.max, ALU.mult)
                else:
                    nc.vector.tensor_tensor(hj, so[:], si[:], ALU.mult)

            def global_groups(l, zt, hg):
                """Allocate h tile now; return one closure per j-group."""
                K = 1 if l == 0 else 4
                hcur = hgpool.tile([P, 4 * SB], F16, tag="hg")
                hprev = hg[l - 1] if l else None
                hg[l] = hcur

                def grp(j):
                    def rhs(k, nb):
                        if l == 0:
                            return zt[:, nb * 512:(nb + 1) * 512]
                        return hprev[:, k * SB + nb * 512:
                                     k * SB + (nb + 1) * 512]
                    gate_group(lambda g, k, j=j: wslot("g", l, k, g, j),
                               rhs, K,
                               lambda g, j=j: bcol("g", l, g, j),
                               hcur[:, j * SB:(j + 1) * SB], l == 2)
                return [lambda j=j: grp(j) for j in range(4)]

            def emit_global(l, zt, hg):
                for f in global_groups(l, zt, hg):
                    f()

            def emit_noise(l, zt, hn):
                hcur = hnpool.tile([P, SB], F16, tag="hn")
                src_t = zt if l == 0 else hn[0]

                def rhs(k, nb):
                    return src_t[:, nb * 512:(nb + 1) * 512]
                gate_group(lambda g, k: wslot("n", l, 0, g, 0),
                           rhs, 1,
                           lambda g: bcol("n", l, g, 0),
                           hcur[:], l == 1)
                hn[l] = hcur

            def emit_head(b, gt, hnf):
                """mu/st rows for superblock b into rowacc[0:2]."""
                for nb in range(NBK):
                    ph = hpool.tile([2, 512], F32, tag="head")
                    for j in range(4):
                        nc.tensor.matmul(
                            ph[0:2, :] if j == 0 else ph[0:1, :],
                            wsum0z if j == 0 else wsum_cols[j],
                            gt[:, j * SB + nb * 512: j * SB + (nb + 1) * 512],
                            start=(j == 0), stop=False, skip_group_check=True)
                    nc.tensor.matmul(ph[0:2, :], aw2,
                                     hnf[:, nb * 512:(nb + 1) * 512],
                                     start=False, stop=True,
                                     skip_group_check=True)
                    nc.vector.tensor_copy(
                        rowacc[0:2, b * SB + nb * 512: b * SB + (nb + 1) * 512],
                        ph[:])

            # Software pipeline: SB b+1's ACT-heavy front groups (g0/n0)
            # are interleaved group-by-group with the PE-heavy g2(b)
            # groups, so the shared psum ring alternates fat-MM and
            # thin-MM steps and both engines stay fed. head(b-1) rides
            # along once its inputs' chains have long drained.
            def start_front(b):
                zt = ztpool.tile([P, SB], F16, tag="zt")
                nc.sync.dma_start_transpose(
                    zt[:], x[b * SB:(b + 1) * SB, :])
                hg = [None, None, None]
                hn = [None, None]
                return zt, hg, hn

            state = {0: start_front(0)}
            zt0, hg0_, hn0_ = state[0]
            emit_global(0, zt0, hg0_)
            emit_noise(0, zt0, hn0_)
            prev = None  # (b, gt, hnf) pending head
            for b in range(nsb):
                zt, hg, hn = state.pop(b)
                emit_global(1, zt, hg)
                emit_noise(1, zt, hn)
                if prev is not None:
                    emit_head(*prev)
                if b + 1 < nsb:
                    nxt = start_front(b + 1)
                    state[b + 1] = nxt
                    emit_global(0, nxt[0], nxt[1])
                    emit_noise(0, nxt[0], nxt[2])
                emit_global(2, zt, hg)
                prev = (b, hg[2], hn[1])
            emit_head(*prev)

            # ---- epilogue: reshape rows to [P, outw], bias + softplus ----
            nc.sync.dma_start(scr_mu.rearrange("(o c) -> o c", o=1),
                              rowacc[0:1, :])
            nc.sync.dma_start(scr_st.rearrange("(o c) -> o c", o=1),
                              rowacc[1:2, :])
            muT = cpool.tile([P, outw], F32)
            nc.sync.dma_start(muT[:], scr_mu.rearrange("(p c) -> p c", p=P))
            stT = cpool.tile([P, outw], F32)
            nc.sync.dma_start(stT[:], scr_st.rearrange("(p c) -> p c", p=P))
            nc.vector.tensor_scalar_add(muT[:], muT[:], b_sum)
            nc.sync.dma_start(mu_d, muT[:])
            nc.scalar.activation(stT[:], stT[:], AF.Exp, bias=a_bias)
            nc.scalar.activation(stT[:], stT[:], AF.Ln, bias=1.0)
            nc.vector.tensor_scalar_add(stT[:], stT[:], 1e-6)
            nc.sync.dma_start(sg_d, stT[:])

    nc.compile()
    return nc


def _unshuffle(arr, tok):
    """[P, tok//P] device layout -> flat [tok] token order."""
    return np.asarray(arr).reshape(tok)


def run(X, weights_kwargs, tok_per_core, n_cores, trace=False):
    """Shard X over cores, run, gather. X: [nt, T, IN] with nt*T == n_cores*tok."""
    wt, bias, b_sum, a_bias = pack_host(**weights_kwargs)
    nc = build_nc(tok_per_core, b_sum, a_bias)
    Xf = np.asarray(X).reshape(-1, IN).astype(np.float16)
    in_maps = []
    for c in range(n_cores):
        shard = Xf[c * tok_per_core:(c + 1) * tok_per_core]
        in_maps.append({"x": np.ascontiguousarray(shard),
                        "wt": wt, "bias": bias})
    res = bass_utils.run_bass_kernel_spmd(
        nc, in_maps, core_ids=list(range(n_cores)), trace=trace)
    mus, sgs = [], []
    for c in range(n_cores):
        mus.append(_unshuffle(res.results[c]["mu"], tok_per_core))
        sgs.append(_unshuffle(res.results[c]["sigma"], tok_per_core))
    nt = X.shape[0]
    mu = np.concatenate(mus).reshape(nt, T).astype(np.float32)
    sg = np.concatenate(sgs).reshape(nt, T).astype(np.float32)
    return mu, sg, res


def kernel(X, g_Wih0, g_bih0, g_bhh0, g_Wih, g_bih, g_bhh, e_W, e_b,
           n_Wih0, n_bih0, n_bhh0, n_Wih, n_bih, n_bhh, a_W, a_b):
    wk = dict(g_Wih0=g_Wih0, g_bih0=g_bih0, g_bhh0=g_bhh0, g_Wih=g_Wih,
              g_bih=g_bih, g_bhh=g_bhh, e_W=e_W, e_b=e_b, n_Wih0=n_Wih0,
              n_bih0=n_bih0, n_bhh0=n_bhh0, n_Wih=n_Wih, n_bih=n_bih,
              n_bhh=n_bhh, a_W=a_W, a_b=a_b)
    tok = FULL_NT * T // NCORES      # 24576
    mu, sg, _ = run(X, wk, tok, NCORES)
    return mu, sg
